# revision 1
# baseline (speedup 1.0000x reference)
"""Per-pixel kernel-lookup conv for trn2, data-parallel over batch on 8 cores.

Per core (one image): conv against all 128 kernels via 3 shifted fp16 matmuls
(K=48 = 16 channels x 3 dy rows), then a fused DVE select
(mask = (idx == j)) * conv, then a ones-matmul partition-reduce, ACT evac.
"""
import numpy as np

RAST = 126 * 128  # output raster, 126 rows padded to 128 wide
_NC_CACHE = {}


def _split_waits_json(bj: bytes) -> bytes:
    """Walrus rejects >4 sync-waits per instruction (and ~2 on Matmult).
    Split excess waits onto same-engine NoOps inserted just before."""
    import json

    j = json.loads(bj)
    ctr = 0
    for f in j["functions"]:
        for bb in f["blocks"]:
            out = []
            for inst in bb["instructions"]:
                si = inst.get("sync_info")
                cap = 1
                waits = (si or {}).get("on_wait") or []
                if len(waits) > cap:
                    extra, keep = waits[:-cap], waits[-cap:]
                    for g in range(0, len(extra), 1):
                        ctr += 1
                        out.append({
                            "debug": inst.get("debug", 0),
                            "engine": inst["engine"],
                            "ins": [],
                            "name": f"WS-{ctr}",
                            "opcode": "NoOp",
                            "outs": [],
                            "sync_info": {"on_update": [],
                                          "on_wait": extra[g:g + 1]},
                        })
                    si["on_wait"] = keep
                out.append(inst)
            bb["instructions"] = out
    return json.dumps(j).encode()


def _build_nc():
    from contextlib import ExitStack

    import concourse.bass as bass
    import concourse.tile as tile
    from concourse import mybir

    F32 = mybir.dt.float32
    F16 = mybir.dt.float16

    nc = bass.Bass(trn_type="TRN2", target_bir_lowering=False)
    d = nc.dram_tensor("d", [16, 128, 128], F16, kind="ExternalInput")
    idxb = nc.dram_tensor("idxb", [128, RAST], F16, kind="ExternalInput")
    wt = nc.dram_tensor("wt", [48, 384], F16, kind="ExternalInput")
    iotain = nc.dram_tensor("iotain", [128, 1], F32, kind="ExternalInput")
    o = nc.dram_tensor("o", [1, RAST], F32, kind="ExternalOutput")

    with tile.TileContext(nc) as tc, ExitStack() as ctx:
        sb = ctx.enter_context(tc.tile_pool(name="sb", bufs=1))
        msk = ctx.enter_context(tc.tile_pool(name="msk", bufs=3))
        psc_pool = ctx.enter_context(tc.tile_pool(name="psc", bufs=4, space="PSUM"))
        pso_pool = ctx.enter_context(tc.tile_pool(name="pso", bufs=2, space="PSUM"))

        iota_f = sb.tile([128, 1], F32)
        nc.sync.dma_start(iota_f[:], iotain.ap())
        ones = sb.tile([128, 1], F16)
        nc.vector.memset(ones[:], 1.0)
        wt_t = sb.tile([48, 384], F16)
        nc.sync.dma_start(wt_t[:], wt.ap())

        # buf[dy*16+c, h*128+w] = data[c, h+dy, w]; 512-col zero pad for the
        # dx-shifted reads of the last chunk.
        buf = sb.tile([48, RAST + 512], F16)
        nc.vector.memset(buf[:, RAST:], 0.0)
        for dy in range(3):
            for h0 in range(0, 126, 28):
                h1 = min(h0 + 28, 126)
                nc.sync.dma_start(
                    buf[dy * 16:(dy + 1) * 16, h0 * 128:h1 * 128],
                    d.ap()[:, dy + h0:dy + h1, :])

        idx_t = sb.tile([128, RAST], F16)
        for q in range(8):
            nc.sync.dma_start(idx_t[:, q * 2016:(q + 1) * 2016],
                              idxb.ap()[:, q * 2016:(q + 1) * 2016])

        out_sb = sb.tile([1, RAST], F32)

        NCH = (RAST + 511) // 512  # 32 chunks
        pso = None
        for c in range(NCH):
            n0 = c * 512
            ncols = min(512, RAST - n0)
            psc = psc_pool.tile([128, 512], F32)
            for dx in range(3):
                nc.tensor.matmul(
                    psc[:, :ncols],
                    lhsT=wt_t[:, dx * 128:(dx + 1) * 128],
                    rhs=buf[:, n0 + dx:n0 + dx + ncols],
                    start=(dx == 0), stop=(dx == 2),
                )
            m = msk.tile([128, 512], F16)
            nc.vector.scalar_tensor_tensor(
                out=m[:, :ncols], in0=idx_t[:, n0:n0 + ncols],
                scalar=iota_f[:], in1=psc[:, :ncols],
                op0=mybir.AluOpType.is_equal, op1=mybir.AluOpType.mult,
            )
            if c % 2 == 0:
                pso = pso_pool.tile([1, 1024], F32)
            off = (c % 2) * 512
            nc.tensor.matmul(pso[:, off:off + ncols], lhsT=ones[:],
                             rhs=m[:, :ncols], start=True, stop=True)
            if c % 2 == 1 or c == NCH - 1:
                g0 = (c // 2) * 1024
                gcols = n0 + ncols - g0
                nc.scalar.copy(out_sb[0:1, g0:g0 + gcols], pso[0:1, 0:gcols])

        for q in range(16):
            nc.sync.dma_start(o.ap()[:, q * 1008:(q + 1) * 1008],
                              out_sb[0:1, q * 1008:(q + 1) * 1008])

    orig = nc.to_json_bytes
    nc.to_json_bytes = lambda: _split_waits_json(orig())
    return nc


def _get_nc():
    if "nc" not in _NC_CACHE:
        _NC_CACHE["nc"] = _build_nc()
    return _NC_CACHE["nc"]


def _in_maps(data, kernel_idx, weights):
    B = data.shape[0]
    # wt[dy*16+c, dx*128+j] = weights[j, c, dy, dx]
    wt2 = np.ascontiguousarray(
        np.transpose(weights, (2, 1, 3, 0)).reshape(48, 384)
    ).astype(np.float16)
    iota = np.arange(128, dtype=np.float32).reshape(128, 1)
    maps = []
    for b in range(B):
        idxr = np.full((126, 128), 500.0, dtype=np.float32)
        idxr[:, :126] = kernel_idx[b].astype(np.float32)
        idxb = np.ascontiguousarray(
            np.broadcast_to(idxr.reshape(1, RAST), (128, RAST))
        ).astype(np.float16)
        maps.append({
            "d": data[b].astype(np.float16),
            "idxb": idxb,
            "wt": wt2,
            "iotain": iota,
        })
    return maps


def kernel(data, kernel_idx, weights, _trace=False):
    from concourse.bass_utils import run_bass_kernel_spmd

    data = np.asarray(data, dtype=np.float32)
    kernel_idx = np.asarray(kernel_idx)
    weights = np.asarray(weights, dtype=np.float32)
    B = data.shape[0]
    nc = _get_nc()
    res = run_bass_kernel_spmd(nc, _in_maps(data, kernel_idx, weights),
                               core_ids=list(range(B)), trace=_trace)
    out = np.stack([r["o"].reshape(126, 128)[:, :126] for r in res.results])
    if _trace:
        return out.astype(np.float32), res
    return out.astype(np.float32)

